# revision 19
# baseline (speedup 1.0000x reference)
"""DLinear fused kernel for 8 TRN2 NeuronCores.

Math: the whole module is linear in x.
  out[b,n,:] = sum_c wf_c * ( x[b,c,n,:] @ (Ws + (Wt-Ws)@A)^T ) + bias
  bias = sum(wf) * (bs + bt) + bf,  A = edge-padded moving-average matrix.

Device pipeline (per core, 8 batches = 4096 rows, 4 bb blocks of 1024):
  - x is quantized per channel to int8 on host with kappa-matched scales
    (wf_ch * s_ch == kappa); kappa folds into the bf16 weights
    (weights-only host math).  The device does the channel reduction and
    the matmul.
  - channel combine, two transports (HW-measured costs drove the mix):
      A-tiles: raw int8 over SP HWDGE (1 B/elem both DMA sides),
        combined by two DVE mixed adds (int8 operands force DVE 1x
        mode, ~1.14 us per [128,1024] add).
      B-tiles: SWDGE cast DMA (int8 HBM -> bf16 SBUF, 2 B/elem on the
        write side), combined by two DVE bf16 adds (2x mode, ~0.6 us).
    12 A / 4 B equalizes DMA-queue seconds against DVE seconds (the
    exchange rate is ~1.1 queue-us per DVE-us for every transport).
    Pool/GpSimd tensor ops are banned: HW-measured, a running Pool op
    ~2.5x-slows both DVE and PE via SBUF interference.  DMA cast+accum
    chains were tried and rejected: read-modify-write doubles queue
    cost and >4 KB hops read stale data.
  - DVE program order is arrival order, not bb order: bb0's and bb3's
    B-cast adds are emitted early/mid-stream so late A-transfers never
    head-of-line-block the DVE FIFO.
  - matmul weights-stationary bf16, k-inner per (bb, h, pc) for the
    middle bbs; bb0 and bb3 run k-OUTER across all 6 (h, pc) PSUM
    tiles.  bb3's k-rows are ordered [B, B, A, A] = arrival order, so
    after the final input lands only one 6-matmul k-row remains.
  - each PSUM tile drains right after its k=3 matmul (fused bias add on
    ScalarE); per-h 344 KB output DMAs (6 KB rows) on the ACT HWDGE
    ring overlap the other half's drains.
"""

import numpy as np
import ml_dtypes

import concourse.bacc as bacc
import concourse.mybir as mybir
import concourse.tile as tile
from concourse.bass_utils import run_bass_kernel_spmd

N_CORES = 8
B, C, N, L, P = 64, 3, 512, 512, 336
KERNEL_W, PAD = 25, 12
BPC = B // N_CORES          # batches per core = 8
BB = 4                      # row blocks per core (1024 rows each)
NH, HW = 2, 512             # halves per block, rows per half
RB = NH * HW                # rows per block = 1024
LC = 4                      # l chunks of 128
PC, PCW = 3, 112            # p chunks x width (3*112 = 336)

# A-tiles (raw int8 + DVE mixed adds) per bb; the rest are B-tiles
# (SWDGE cast + DVE bf16 adds).  bb0/bb3 keep 2 B-tiles each so the
# ramp and the tail are fed from both rings in parallel.
NA_BB = {0: 2, 1: 4, 2: 4, 3: 2}
# lc indices carried by (A-slots, B-slots) per bb: bb3's B-slots take
# the LOW k-rows so its k-outer consumption matches arrival order.
def _lc_split(bb):
    na = NA_BB[bb]
    if bb == BB - 1:
        return tuple(range(LC - na, LC)), tuple(range(LC - na))
    return tuple(range(na)), tuple(range(na, LC))

BF16 = mybir.dt.bfloat16
F32 = mybir.dt.float32
I8 = mybir.dt.int8
OUT_DT = BF16

LAST_RESULT = None
_CACHE = {}


def _movavg_matrix():
    A = np.zeros((L, L), np.float64)
    for lp in range(L):
        for kk in range(lp - PAD, lp + PAD + 1):
            A[lp, min(max(kk, 0), L - 1)] += 1.0 / KERNEL_W
    return A


def _build():
    nc = bacc.Bacc("TRN2", target_bir_lowering=False, debug=False)
    n_a = sum(NA_BB.values())
    n_b = BB * LC - n_a
    a_off, b_off = {}, {}
    oa = ob = 0
    for bb in range(BB):
        a_off[bb], b_off[bb] = oa, ob
        oa += NA_BB[bb]
        ob += LC - NA_BB[bb]
    xa_d = nc.dram_tensor("xa", (n_a, 128, C * RB), I8, kind="ExternalInput")
    xb_d = nc.dram_tensor("xb", (n_b, 128, C * RB), I8, kind="ExternalInput")
    w_d = nc.dram_tensor("w", (LC, 128, P), BF16, kind="ExternalInput")
    b_d = nc.dram_tensor("bias", (PCW, PC), F32, kind="ExternalInput")
    o_d = nc.dram_tensor("o", (BB, PCW, NH, PC, HW), OUT_DT, kind="ExternalOutput")

    with tile.TileContext(nc) as tc:
        with (
            tc.tile_pool(name="const", bufs=1) as constp,
            tc.tile_pool(name="xin", bufs=3) as xinp,
            tc.tile_pool(name="xb", bufs=2) as xbp,
            tc.tile_pool(name="tp", bufs=2) as tpp,
            tc.tile_pool(name="xcp", bufs=3) as xcp,
            tc.tile_pool(name="ps", bufs=8, space="PSUM") as psp,
            tc.tile_pool(name="ostage", bufs=3) as osp,
        ):
            wts = []
            for k in range(LC):
                wt = constp.tile([128, P], BF16, tag=f"w{k}", name=f"w{k}")
                nc.scalar.dma_start(wt[:], w_d[k])
                wts.append(wt)
            btile = constp.tile([PCW, PC], F32, tag="bias", name="bias")
            nc.scalar.dma_start(btile[:], b_d[:])

            # xcs[bb][k] = combined moving tile for physical lc chunk k
            xcs = {bb: [None] * LC for bb in range(BB)}

            def emit_a_dma(bb):
                tiles = []
                for i in range(NA_BB[bb]):
                    xf = xinp.tile([128, C * RB], I8, tag=f"xa{i}",
                                   name=f"xa{i}_{bb}")
                    nc.sync.dma_start(xf[:], xa_d[a_off[bb] + i])
                    tiles.append(xf)
                return tiles

            def emit_b_dma(bb):
                tiles = []
                for i in range(LC - NA_BB[bb]):
                    xf = xbp.tile([128, C * RB], BF16, tag=f"xb{i}",
                                  name=f"xb{i}_{bb}")
                    nc.gpsimd.dma_start(xf[:], xb_d[b_off[bb] + i])
                    tiles.append(xf)
                return tiles

            def emit_adds(bb, tiles, lcs, kind):
                for xf, lc in zip(tiles, lcs):
                    t = tpp.tile([128, RB], BF16, tag=f"t{kind}{lc}",
                                 name=f"t{kind}{lc}_{bb}")
                    nc.vector.tensor_add(t[:], xf[:, 0:RB], xf[:, RB:2 * RB])
                    xc = xcp.tile([128, RB], BF16, tag=f"xc{lc}",
                                  name=f"xc{lc}_{bb}")
                    nc.vector.tensor_add(xc[:], t[:], xf[:, 2 * RB:3 * RB])
                    xcs[bb][lc] = xc

            def emit_mm(bb):
                ost = osp.tile([PCW, NH * PC * HW], OUT_DT, tag="ost",
                               name=f"ost{bb}")
                if bb in (0, BB - 1):
                    pss = [[psp.tile([PCW, HW], F32, tag="ps",
                                     name=f"ps{bb}_{h}_{pc}")
                            for pc in range(PC)] for h in range(NH)]
                    for k in range(LC):
                        for h in range(NH):
                            for pc in range(PC):
                                nc.tensor.matmul(
                                    pss[h][pc][:],
                                    wts[k][:, pc * PCW:(pc + 1) * PCW],
                                    xcs[bb][k][:, h * HW:(h + 1) * HW],
                                    start=(k == 0),
                                    stop=(k == LC - 1),
                                )
                                if k == LC - 1:
                                    off = (h * PC + pc) * HW
                                    nc.scalar.activation(
                                        ost[:, off:off + HW],
                                        pss[h][pc][:],
                                        mybir.ActivationFunctionType.Identity,
                                        bias=btile[:, pc:pc + 1],
                                    )
                                    if pc == PC - 1:
                                        nc.scalar.dma_start(
                                            o_d[bb, :, h],
                                            ost[:, h * PC * HW:
                                                 (h + 1) * PC * HW])
                else:
                    for h in range(NH):
                        for pc in range(PC):
                            ps = psp.tile([PCW, HW], F32, tag="ps",
                                          name=f"ps{bb}_{h}_{pc}")
                            for k in range(LC):
                                nc.tensor.matmul(
                                    ps[:],
                                    wts[k][:, pc * PCW:(pc + 1) * PCW],
                                    xcs[bb][k][:, h * HW:(h + 1) * HW],
                                    start=(k == 0),
                                    stop=(k == LC - 1),
                                )
                            nc.scalar.activation(
                                ost[:, (h * PC + pc) * HW:
                                     (h * PC + pc + 1) * HW],
                                ps[:],
                                mybir.ActivationFunctionType.Identity,
                                bias=btile[:, pc:pc + 1],
                            )
                        nc.scalar.dma_start(
                            o_d[bb, :, h],
                            ost[:, h * PC * HW:(h + 1) * PC * HW])

            a_lcs = {bb: _lc_split(bb)[0] for bb in range(BB)}
            b_lcs = {bb: _lc_split(bb)[1] for bb in range(BB)}

            # --- global emission order (per-engine FIFOs follow it) ---
            a0 = emit_a_dma(0)
            b0 = emit_b_dma(0)
            b3 = emit_b_dma(3)          # bb3's casts issue early on SWDGE
            emit_adds(0, a0, a_lcs[0], "a")
            emit_adds(0, b0, b_lcs[0], "b")
            a1 = emit_a_dma(1)
            emit_mm(0)
            emit_adds(1, a1, a_lcs[1], "a")
            a2 = emit_a_dma(2)
            emit_adds(3, b3, b_lcs[3], "b")   # mid-stream, before bb2 adds
            emit_mm(1)
            emit_adds(2, a2, a_lcs[2], "a")
            a3 = emit_a_dma(3)
            emit_mm(2)
            emit_adds(3, a3, a_lcs[3], "a")
            emit_mm(3)

    nc.compile()
    return nc


def kernel(x, Ws, bs, Wt, bt, Wf, bf):
    global LAST_RESULT
    # ---- host-side weight folding (f64, weights only) ----
    A = _movavg_matrix()
    Weff = Ws.astype(np.float64) + (Wt.astype(np.float64) - Ws.astype(np.float64)) @ A
    wf = Wf[0].astype(np.float64)                      # (3,)

    # ---- kappa-matched per-channel int8 quantization ----
    am = np.array([np.abs(x[:, ch]).max() for ch in range(C)], np.float64)
    am = np.maximum(am, 1e-30)
    kappa = float((np.abs(wf) * am).max()) / 127.0
    if kappa == 0.0:
        kappa = 1.0
    s = kappa / np.where(wf == 0, np.inf, wf)          # signed scales
    Wp = kappa * Weff                                  # (336, 512)
    WT = np.ascontiguousarray(Wp.T).reshape(LC, 128, P).astype(ml_dtypes.bfloat16)
    bias = wf.sum() * (bs.astype(np.float64) + bt.astype(np.float64)) + float(bf[0])
    bias_r = np.ascontiguousarray(bias.astype(np.float32).reshape(PC, PCW).T)

    # ---- build / compile (cached; kernel is data-independent) ----
    if "nc" not in _CACHE:
        _CACHE["nc"] = _build()
    nc = _CACHE["nc"]

    # ---- host-side quantize + sharding / layout ----
    xq = np.empty(x.shape, np.int8)
    for ch in range(C):
        xq[:, ch] = np.clip(np.round(x[:, ch] * np.float64(1.0 / s[ch])), -127, 127)
    # [core, bb, lc, p, c, h, n] -> (core, BB, LC, 128, C*1024)
    xr = xq.reshape(N_CORES, BB, NH, C, N, LC, 128)
    xr = xr.transpose(0, 1, 5, 6, 3, 2, 4)
    xr = np.ascontiguousarray(xr.reshape(N_CORES, BB, LC, 128, C * RB))

    in_maps = []
    for i in range(N_CORES):
        xa = np.concatenate([xr[i, bb, list(_lc_split(bb)[0])]
                             for bb in range(BB)])
        xb = np.concatenate([xr[i, bb, list(_lc_split(bb)[1])]
                             for bb in range(BB) if NA_BB[bb] < LC])
        in_maps.append({
            "xa": np.ascontiguousarray(xa),
            "xb": np.ascontiguousarray(xb),
            "w": WT,
            "bias": bias_r,
        })

    res = run_bass_kernel_spmd(nc, in_maps, core_ids=list(range(N_CORES)))
    LAST_RESULT = res

    # ---- gather / unshard ----
    outs = []
    for i in range(N_CORES):
        o = res.results[i]["o"].astype(np.float32)     # (BB, 112, NH, PC, 512)
        o = o.transpose(0, 2, 4, 3, 1).reshape(BPC, N, P)
        outs.append(o)
    out = np.stack(outs).reshape(B, N, P)[:, None]     # (64, 1, 512, 336)
    return out.astype(np.float32)


# revision 22
# speedup vs baseline: 1.0130x; 1.0130x over previous
"""DLinear fused kernel for 8 TRN2 NeuronCores.

Math: the whole module is linear in x.
  out[b,n,:] = sum_c wf_c * ( x[b,c,n,:] @ (Ws + (Wt-Ws)@A)^T ) + bias
  bias = sum(wf) * (bs + bt) + bf,  A = edge-padded moving-average matrix.

Device pipeline (per core, 8 batches = 4096 rows, 4 bb blocks of 1024):
  - x is quantized per channel to int8 on host with kappa-matched scales
    (wf_ch * s_ch == kappa); kappa folds into the bf16 weights
    (weights-only host math).  The device does the channel reduction and
    the matmul.
  - channel combine, two transports (HW-measured costs drove the mix):
      A-tiles: raw int8 over SP HWDGE (1 B/elem both DMA sides),
        combined by two DVE mixed adds (int8 operands force DVE 1x
        mode, ~1.14 us per [128,1024] add).
      B-tiles: SWDGE cast DMA (int8 HBM -> bf16 SBUF, 2 B/elem on the
        write side), combined by two DVE bf16 adds (2x mode, ~0.6 us).
    12 A / 4 B equalizes DMA-queue seconds against DVE seconds (the
    exchange rate is ~1.1 queue-us per DVE-us for every transport).
    Pool/GpSimd tensor ops are banned: HW-measured, a running Pool op
    ~2.5x-slows both DVE and PE via SBUF interference.  DMA cast+accum
    chains were tried and rejected: read-modify-write doubles queue
    cost and >4 KB hops read stale data.
  - DVE program order is arrival order, not bb order: bb0's and bb3's
    B-cast adds are emitted early/mid-stream so late A-transfers never
    head-of-line-block the DVE FIFO.
  - matmul weights-stationary bf16, k-inner per (bb, h, pc) for the
    middle bbs; bb0 and bb3 run k-OUTER across all 6 (h, pc) PSUM
    tiles.  bb3's k-rows are ordered [B, B, A, A] = arrival order, so
    after the final input lands only one 6-matmul k-row remains.
  - each PSUM tile drains right after its k=3 matmul (fused bias add on
    ScalarE); per-h 344 KB output DMAs (6 KB rows) on the ACT HWDGE
    ring overlap the other half's drains.
"""

import numpy as np
import ml_dtypes

import concourse.bacc as bacc
import concourse.mybir as mybir
import concourse.tile as tile
from concourse.bass_utils import run_bass_kernel_spmd

N_CORES = 8
B, C, N, L, P = 64, 3, 512, 512, 336
KERNEL_W, PAD = 25, 12
BPC = B // N_CORES          # batches per core = 8
BB = 4                      # row blocks per core (1024 rows each)
NH, HW = 2, 512             # halves per block, rows per half
RB = NH * HW                # rows per block = 1024
LC = 4                      # l chunks of 128
PC, PCW = 3, 112            # p chunks x width (3*112 = 336)

# Per-bb transport of the 4 lc tiles, in k order.  Three transports
# balance three resources at ~29 us each:
#   A: raw int8 ship + 2 DVE mixed adds   (queue 1.2, DVE 2.27)
#   T: raw int8 ship + 3 ACT casts + 2 DVE bf16 adds (q 1.2, DVE 1.2,
#      ACT 2.3 -- ScalarE has slack beyond its PSUM drains)
#   B: SWDGE cast ship + 2 DVE bf16 adds  (queue 2.13, DVE 1.2)
# bb3's k-rows are ordered so the last-arriving raw tiles are the last
# k-rows of its k-outer loop (minimal post-input matmul tail).
KIND_BB = {0: "ATAB", 1: "AATA", 2: "ATAT", 3: "BTAA"}

BF16 = mybir.dt.bfloat16
F32 = mybir.dt.float32
I8 = mybir.dt.int8
OUT_DT = BF16

LAST_RESULT = None
_CACHE = {}


def _movavg_matrix():
    A = np.zeros((L, L), np.float64)
    for lp in range(L):
        for kk in range(lp - PAD, lp + PAD + 1):
            A[lp, min(max(kk, 0), L - 1)] += 1.0 / KERNEL_W
    return A


def _raw_slots(bb):
    return [k for k in range(LC) if KIND_BB[bb][k] in "AT"]


def _b_slots(bb):
    return [k for k in range(LC) if KIND_BB[bb][k] == "B"]


def _build():
    nc = bacc.Bacc("TRN2", target_bir_lowering=False, debug=False)
    r_off, b_off = {}, {}
    orr = ob = 0
    for bb in range(BB):
        r_off[bb], b_off[bb] = orr, ob
        orr += len(_raw_slots(bb))
        ob += len(_b_slots(bb))
    xr_d = nc.dram_tensor("xr", (orr, 128, C * RB), I8, kind="ExternalInput")
    xb_d = nc.dram_tensor("xb", (ob, 128, C * RB), I8, kind="ExternalInput")
    w_d = nc.dram_tensor("w", (LC, 128, P), BF16, kind="ExternalInput")
    b_d = nc.dram_tensor("bias", (PCW, PC), F32, kind="ExternalInput")
    o_d = nc.dram_tensor("o", (BB, PCW, NH, PC, HW), OUT_DT, kind="ExternalOutput")

    with tile.TileContext(nc) as tc:
        with (
            tc.tile_pool(name="const", bufs=1) as constp,
            tc.tile_pool(name="xin", bufs=3) as xinp,
            tc.tile_pool(name="xbp", bufs=2) as xbp,
            tc.tile_pool(name="xtp", bufs=2) as xtp,
            tc.tile_pool(name="tp", bufs=2) as tpp,
            tc.tile_pool(name="xcp", bufs=3) as xcp,
            tc.tile_pool(name="ps", bufs=8, space="PSUM") as psp,
            tc.tile_pool(name="ostage", bufs=3) as osp,
        ):
            wts = []
            for k in range(LC):
                wt = constp.tile([128, P], BF16, tag=f"w{k}", name=f"w{k}")
                nc.scalar.dma_start(wt[:], w_d[k])
                wts.append(wt)
            btile = constp.tile([PCW, PC], F32, tag="bias", name="bias")
            nc.scalar.dma_start(btile[:], b_d[:])

            for bb in range(BB):
                kinds = KIND_BB[bb]
                xcs = [None] * LC
                # ---- ship ----
                raw, bcast = {}, {}
                for i, k in enumerate(_raw_slots(bb)):
                    xf = xinp.tile([128, C * RB], I8, tag=f"xr{i}",
                                   name=f"xr{i}_{bb}")
                    nc.sync.dma_start(xf[:], xr_d[r_off[bb] + i])
                    raw[k] = xf
                for i, k in enumerate(_b_slots(bb)):
                    xf = xbp.tile([128, C * RB], BF16, tag=f"xb{i}",
                                  name=f"xb{i}_{bb}")
                    nc.gpsimd.dma_start(xf[:], xb_d[b_off[bb] + i])
                    bcast[k] = xf
                # ---- widen + combine ----
                for k in range(LC):
                    kind = kinds[k]
                    if kind == "T":
                        # ScalarE widens the whole raw tile in one pass
                        xt = xtp.tile([128, C * RB], BF16, tag=f"xt{k % 2}",
                                      name=f"xt{k}_{bb}")
                        nc.scalar.copy(xt[:], raw[k][:])
                        src = xt
                    elif kind == "B":
                        src = bcast[k]
                    else:
                        src = raw[k]
                    t = tpp.tile([128, RB], BF16, tag=f"t{k}",
                                 name=f"t{k}_{bb}")
                    nc.vector.tensor_add(t[:], src[:, 0:RB], src[:, RB:2 * RB])
                    xc = xcp.tile([128, RB], BF16, tag=f"xc{k}",
                                  name=f"xc{k}_{bb}")
                    nc.vector.tensor_add(xc[:], t[:], src[:, 2 * RB:3 * RB])
                    xcs[k] = xc

                # ---- matmul + drain + output ----
                ost = osp.tile([PCW, NH * PC * HW], OUT_DT, tag="ost",
                               name=f"ost{bb}")

                def drain(h, pc, ps):
                    off = (h * PC + pc) * HW
                    if bb == BB - 1 and h == NH - 1:
                        # tail drains on DVE (idle by then); ACT handles h0
                        nc.vector.tensor_scalar_add(
                            ost[:, off:off + HW], ps[:], btile[:, pc:pc + 1])
                    else:
                        nc.scalar.activation(
                            ost[:, off:off + HW], ps[:],
                            mybir.ActivationFunctionType.Identity,
                            bias=btile[:, pc:pc + 1])
                    if pc == PC - 1:
                        nc.scalar.dma_start(
                            o_d[bb, :, h],
                            ost[:, h * PC * HW:(h + 1) * PC * HW])

                if bb in (0, BB - 1):
                    pss = [[psp.tile([PCW, HW], F32, tag="ps",
                                     name=f"ps{bb}_{h}_{pc}")
                            for pc in range(PC)] for h in range(NH)]
                    for k in range(LC):
                        for h in range(NH):
                            for pc in range(PC):
                                nc.tensor.matmul(
                                    pss[h][pc][:],
                                    wts[k][:, pc * PCW:(pc + 1) * PCW],
                                    xcs[k][:, h * HW:(h + 1) * HW],
                                    start=(k == 0),
                                    stop=(k == LC - 1),
                                )
                                if k == LC - 1:
                                    drain(h, pc, pss[h][pc])
                else:
                    for h in range(NH):
                        for pc in range(PC):
                            ps = psp.tile([PCW, HW], F32, tag="ps",
                                          name=f"ps{bb}_{h}_{pc}")
                            for k in range(LC):
                                nc.tensor.matmul(
                                    ps[:],
                                    wts[k][:, pc * PCW:(pc + 1) * PCW],
                                    xcs[k][:, h * HW:(h + 1) * HW],
                                    start=(k == 0),
                                    stop=(k == LC - 1),
                                )
                            drain(h, pc, ps)

    nc.compile()
    return nc


def kernel(x, Ws, bs, Wt, bt, Wf, bf):
    global LAST_RESULT
    # ---- host-side weight folding (f64, weights only) ----
    A = _movavg_matrix()
    Weff = Ws.astype(np.float64) + (Wt.astype(np.float64) - Ws.astype(np.float64)) @ A
    wf = Wf[0].astype(np.float64)                      # (3,)

    # ---- kappa-matched per-channel int8 quantization ----
    am = np.array([np.abs(x[:, ch]).max() for ch in range(C)], np.float64)
    am = np.maximum(am, 1e-30)
    kappa = float((np.abs(wf) * am).max()) / 127.0
    if kappa == 0.0:
        kappa = 1.0
    s = kappa / np.where(wf == 0, np.inf, wf)          # signed scales
    Wp = kappa * Weff                                  # (336, 512)
    WT = np.ascontiguousarray(Wp.T).reshape(LC, 128, P).astype(ml_dtypes.bfloat16)
    bias = wf.sum() * (bs.astype(np.float64) + bt.astype(np.float64)) + float(bf[0])
    bias_r = np.ascontiguousarray(bias.astype(np.float32).reshape(PC, PCW).T)

    # ---- build / compile (cached; kernel is data-independent) ----
    if "nc" not in _CACHE:
        _CACHE["nc"] = _build()
    nc = _CACHE["nc"]

    # ---- host-side quantize + sharding / layout ----
    xq = np.empty(x.shape, np.int8)
    for ch in range(C):
        xq[:, ch] = np.clip(np.round(x[:, ch] * np.float64(1.0 / s[ch])), -127, 127)
    # [core, bb, lc, p, c, h, n] -> (core, BB, LC, 128, C*1024)
    xr = xq.reshape(N_CORES, BB, NH, C, N, LC, 128)
    xr = xr.transpose(0, 1, 5, 6, 3, 2, 4)
    xr = np.ascontiguousarray(xr.reshape(N_CORES, BB, LC, 128, C * RB))

    in_maps = []
    for i in range(N_CORES):
        xraw = np.concatenate([xr[i, bb, _raw_slots(bb)]
                               for bb in range(BB)])
        xbc = np.concatenate([xr[i, bb, _b_slots(bb)]
                              for bb in range(BB) if _b_slots(bb)])
        in_maps.append({
            "xr": np.ascontiguousarray(xraw),
            "xb": np.ascontiguousarray(xbc),
            "w": WT,
            "bias": bias_r,
        })

    res = run_bass_kernel_spmd(nc, in_maps, core_ids=list(range(N_CORES)))
    LAST_RESULT = res

    # ---- gather / unshard ----
    outs = []
    for i in range(N_CORES):
        o = res.results[i]["o"].astype(np.float32)     # (BB, 112, NH, PC, 512)
        o = o.transpose(0, 2, 4, 3, 1).reshape(BPC, N, P)
        outs.append(o)
    out = np.stack(outs).reshape(B, N, P)[:, None]     # (64, 1, 512, 336)
    return out.astype(np.float32)


# revision 26
# speedup vs baseline: 1.1160x; 1.1017x over previous
"""DLinear fused kernel for 8 TRN2 NeuronCores.

Math: the whole module is linear in x.
  out[b,n,:] = sum_c wf_c * ( x[b,c,n,:] @ (Ws + (Wt-Ws)@A)^T ) + bias
  bias = sum(wf) * (bs + bt) + bf,  A = edge-padded moving-average matrix.

Device pipeline (per core, 8 batches = 4096 rows, 8 half-blocks of 512):
  - x is quantized per channel to int8 on host with kappa-matched scales
    (wf_ch * s_ch == kappa), then cast-DMA'd int8->bf16 by SWDGE
    (nc.gpsimd): HBM reads only 1 B/elem; the SDMA datapath widens to
    bf16 on the SBUF write side (int8 codes are exact in bf16).  kappa
    folds into the bf16 weights (weights-only host compute).
  - channel combine collapses to xc = x'_a + x'_b + x'_c: two
    scalar-free bf16 tensor_add per [128,512] tile on DVE.
  - matmul weights-stationary bf16, k-INNER per (half, pc): dense 4-MM
    accumulation groups (HAM-friendly); each PSUM tile drains right
    after its k=3 matmul (fused bias add on ScalarE) and its 114 KB
    output DMA leaves immediately on the sync ring.
  - half-granular streaming (DMA per (bb, half, k)) halves the
    DMA->combine->matmul phase lag and the pipeline tail.
  - deep output staging (10 tiles) absorbs the slow trickle of output
    DMAs (SDMA lanes round-robin against the input stream) so ACT/PE
    never stall on a free staging tile.
DMA rings: x on SWDGE (gpsimd), weights/bias on ACT HWDGE, outputs on
SP HWDGE — no ring ever stalls another stream.
"""

import numpy as np
import ml_dtypes

import concourse.bacc as bacc
import concourse.mybir as mybir
import concourse.tile as tile
from concourse.bass_utils import run_bass_kernel_spmd

N_CORES = 8
B, C, N, L, P = 64, 3, 512, 512, 336
KERNEL_W, PAD = 25, 12
BPC = B // N_CORES          # batches per core = 8
BN = BPC * N                # rows per core = 4096
BB = 4                      # bn blocks per core
NH, HW = 2, 512             # halves per block, rows per half
LC = 4                      # l chunks of 128
PC, PCW = 3, 112            # p chunks x width (3*112 = 336)

BF16 = mybir.dt.bfloat16
F32 = mybir.dt.float32
I8 = mybir.dt.int8
OUT_DT = BF16

LAST_RESULT = None
_CACHE = {}


def _movavg_matrix():
    A = np.zeros((L, L), np.float64)
    for lp in range(L):
        for kk in range(lp - PAD, lp + PAD + 1):
            A[lp, min(max(kk, 0), L - 1)] += 1.0 / KERNEL_W
    return A


def _build():
    nc = bacc.Bacc("TRN2", target_bir_lowering=False, debug=False)
    # one transfer per (bb, half, k): [128, c*512] int8, contiguous
    x_d = nc.dram_tensor("x", (BB, NH, LC, 128, C * HW), I8, kind="ExternalInput")
    # chunks duplicated for the sync HWDGE ring as raw int8: the ring
    # issues ~2us before SWDGE warms up and int8 moves half the lane
    # bytes of the cast path; the (slower) mixed-dtype combine for these
    # runs in DVE's startup window and spare mid-stream cycles.  All are
    # pre-issued before any output DMA enters the FIFO sync ring.
    # startup-window chunks (DVE is idle before the SWDGE stream starts)
    # plus late-phase chunks (Q7 issue/drain is the binding resource at
    # the stream's end; DVE has slack there)
    EARLY = [(0, 0, 0), (0, 0, 1), (0, 0, 2), (0, 1, 0),
             (2, 1, 0), (3, 0, 0), (3, 0, 1)]
    # mid-stream chunks shipped RAW int8 over the ACT HWDGE ring (idle
    # after the weight loads): halves those chunks' DMA-queue bytes; the
    # mixed-dtype combine costs ~+0.4 us/chunk on DVE, which has slack.
    RAWI = [(1, 0, 1), (1, 0, 3), (1, 1, 1), (1, 1, 3),
            (2, 0, 1), (2, 0, 3), (2, 1, 1), (2, 1, 3)]
    xh_d = nc.dram_tensor("xh", (len(EARLY), 128, C * HW), I8,
                          kind="ExternalInput")
    w_d = nc.dram_tensor("w", (LC, 128, P), BF16, kind="ExternalInput")
    b_d = nc.dram_tensor("bias", (PCW, PC), F32, kind="ExternalInput")
    # [112, pc*512] per (bb, h): 3 KB contiguous rows — 1 KB-row output
    # transfers measured only 16 GB/s/lane vs ~22 for 3 KB rows
    o_d = nc.dram_tensor("o", (BB, NH, PCW, PC, HW), OUT_DT, kind="ExternalOutput")

    with tile.TileContext(nc) as tc:
        with (
            tc.tile_pool(name="const", bufs=1) as constp,
            tc.tile_pool(name="xin", bufs=3) as xinp,
            tc.tile_pool(name="raw", bufs=2) as rawp,
            tc.tile_pool(name="xcp", bufs=3) as xcp,
            tc.tile_pool(name="ps", bufs=6, space="PSUM") as psp,
            # deep output staging: output DMAs trickle slowly (SDMA lanes
            # round-robin against the input stream), so ACT/PE must never
            # wait on a free staging tile
            tc.tile_pool(name="ostage", bufs=10) as osp,
        ):
            wts = []
            for k in range(LC):
                wt = constp.tile([128, P], BF16, tag=f"w{k}", name=f"w{k}")
                nc.scalar.dma_start(wt[:], w_d[k])
                wts.append(wt)
            btile = constp.tile([PCW, PC], F32, tag="bias", name="bias")
            nc.scalar.dma_start(btile[:], b_d[:])

            early_tiles = {}
            for j, key in enumerate(EARLY):
                xe = xinp.tile([128, C * HW], I8, tag=f"xe{j}", name=f"xe{j}")
                nc.sync.dma_start(xe[:], xh_d[j])
                early_tiles[key] = xe

            for bb in range(BB):
                for h in range(NH):
                    last_unit = (bb == BB - 1 and h == NH - 1)
                    ost = osp.tile([PCW, PC * HW], OUT_DT, tag="ost",
                                   name=f"ost{bb}_{h}")
                    if last_unit:
                        # k-OUTER for the final half-block only: its
                        # matmuls run as each chunk arrives, so after the
                        # last input packet just 3 MMs + drains remain
                        # (k-inner would leave all 12).  HAM cost is moot
                        # since the PE is finishing anyway.
                        pss = [psp.tile([PCW, HW], F32, tag="ps",
                                        name=f"ps{bb}_{h}_{pc}")
                               for pc in range(PC)]
                    xcs = []
                    for k in range(LC):
                        if (bb, h, k) in early_tiles:
                            # raw int8 tile; int8 codes are exact in bf16
                            # so the mixed-dtype adds produce identical
                            # values to the cast path
                            xf = early_tiles[(bb, h, k)]
                        elif (bb, h, k) in RAWI:
                            xf = rawp.tile([128, C * HW], I8, tag=f"r{h}{k}",
                                           name=f"r{h}{k}_{bb}")
                            nc.scalar.dma_start(xf[:], x_d[bb, h, k])
                        else:
                            xf = xinp.tile([128, C * HW], BF16, tag=f"x{h}{k}",
                                           name=f"x{h}{k}_{bb}")
                            nc.gpsimd.dma_start(xf[:], x_d[bb, h, k])
                        t = xcp.tile([128, HW], BF16, tag=f"t{h}{k}",
                                     name=f"t{h}{k}_{bb}")
                        nc.vector.tensor_add(t[:], xf[:, 0:HW],
                                             xf[:, HW:2 * HW])
                        xc = xcp.tile([128, HW], BF16, tag=f"xc{h}{k}",
                                      name=f"xc{h}{k}_{bb}")
                        nc.vector.tensor_add(xc[:], t[:], xf[:, 2 * HW:3 * HW])
                        xcs.append(xc)
                        if last_unit:
                            for pc in range(PC):
                                nc.tensor.matmul(
                                    pss[pc][:],
                                    wts[k][:, pc * PCW:(pc + 1) * PCW],
                                    xc[:],
                                    start=(k == 0),
                                    stop=(k == LC - 1),
                                )
                                if k == LC - 1:
                                    nc.scalar.activation(
                                        ost[:, pc * HW:(pc + 1) * HW],
                                        pss[pc][:],
                                        mybir.ActivationFunctionType.Identity,
                                        bias=btile[:, pc:pc + 1],
                                    )
                                    # per-pc output DMA: the final transfer
                                    # on the critical path is 114 KB, not
                                    # 344 KB
                                    nc.sync.dma_start(
                                        o_d[bb, h, :, pc],
                                        ost[:, pc * HW:(pc + 1) * HW])

                    if not last_unit:
                        # dense k-inner accumulation per (half, pc); the
                        # shared wide staging tile's single 344 KB output
                        # DMA leaves after the last pc drain
                        for pc in range(PC):
                            ps = psp.tile([PCW, HW], F32, tag="ps",
                                          name=f"ps{bb}_{h}_{pc}")
                            for k in range(LC):
                                nc.tensor.matmul(
                                    ps[:],
                                    wts[k][:, pc * PCW:(pc + 1) * PCW],
                                    xcs[k][:],
                                    start=(k == 0),
                                    stop=(k == LC - 1),
                                )
                            nc.scalar.activation(
                                ost[:, pc * HW:(pc + 1) * HW],
                                ps[:],
                                mybir.ActivationFunctionType.Identity,
                                bias=btile[:, pc:pc + 1],
                            )
                        nc.sync.dma_start(o_d[bb, h], ost[:])

    nc.compile()
    return nc


def kernel(x, Ws, bs, Wt, bt, Wf, bf):
    global LAST_RESULT
    # ---- host-side weight folding (f64, weights only) ----
    A = _movavg_matrix()
    Weff = Ws.astype(np.float64) + (Wt.astype(np.float64) - Ws.astype(np.float64)) @ A
    wf = Wf[0].astype(np.float64)                      # (3,)

    # ---- kappa-matched per-channel int8 quantization ----
    am = np.array([np.abs(x[:, ch]).max() for ch in range(C)], np.float64)
    am = np.maximum(am, 1e-30)
    kappa = float((np.abs(wf) * am).max()) / 127.0
    if kappa == 0.0:
        kappa = 1.0
    s = kappa / np.where(wf == 0, np.inf, wf)          # signed scales
    Wp = kappa * Weff                                  # (336, 512)
    WT = np.ascontiguousarray(Wp.T).reshape(LC, 128, P).astype(ml_dtypes.bfloat16)
    bias = wf.sum() * (bs.astype(np.float64) + bt.astype(np.float64)) + float(bf[0])
    bias_r = np.ascontiguousarray(bias.astype(np.float32).reshape(PC, PCW).T)

    # ---- build / compile (cached; kernel is data-independent) ----
    if "nc" not in _CACHE:
        _CACHE["nc"] = _build()
    nc = _CACHE["nc"]

    # ---- host-side quantize + sharding / layout ----
    xq = np.empty(x.shape, np.int8)
    for ch in range(C):
        xq[:, ch] = np.clip(np.round(x[:, ch] * np.float64(1.0 / s[ch])), -127, 127)
    xr = xq.reshape(N_CORES, BPC, C, N, L)
    xr = xr.transpose(0, 2, 4, 1, 3)                   # [core, c, l, bl, n]
    xr = xr.reshape(N_CORES, C, LC, 128, BB, NH, HW)
    xr = xr.transpose(0, 4, 5, 2, 3, 1, 6)             # [core, bb, h, lc, 128, c, hw]
    xr = xr.reshape(N_CORES, BB, NH, LC, 128, C * HW)

    in_maps = []
    for i in range(N_CORES):
        xi = np.ascontiguousarray(xr[i])
        xh = np.stack([xi[bb, h, k] for (bb, h, k) in
                       [(0, 0, 0), (0, 0, 1), (0, 0, 2), (0, 1, 0),
                        (2, 1, 0), (3, 0, 0), (3, 0, 1)]])
        in_maps.append({
            "x": xi,
            "xh": np.ascontiguousarray(xh),
            "w": WT,
            "bias": bias_r,
        })

    res = run_bass_kernel_spmd(nc, in_maps, core_ids=list(range(N_CORES)))
    LAST_RESULT = res

    # ---- gather / unshard ----
    outs = []
    for i in range(N_CORES):
        o = res.results[i]["o"].astype(np.float32)     # (BB, NH, 112, PC, 512)
        o = o.transpose(0, 1, 4, 3, 2).reshape(BPC, N, P)
        outs.append(o)
    out = np.stack(outs).reshape(B, N, P)[:, None]     # (64, 1, 512, 336)
    return out.astype(np.float32)

